# revision 15
# baseline (speedup 1.0000x reference)
"""AttentionPooling Trainium2 kernel (8-core data-parallel).

Math per batch row b (B=2048, S=512, D=128):
    keys   = x @ Wk^T + bk + pos @ Wp^T + bp
    scores = (keys . q) * D**-0.5
    w      = softmax(scores)
    out    = sum_s w_s * (x_s @ Wv^T + bv)

Weight folding (host):
    qk = Wk^T q * D**-0.5, qp = Wp^T q * D**-0.5; the bias terms shift every
    score equally and cancel in softmax.  Folding qk INTO x
        xq[b,s,d] = x[b,s,d] * qk[d]       (bf16)
    turns the score into a plain sum over d:
        score[b,s] = sum_d xq[b,s,d] + possum[b,s],  possum = pos @ qp
    and the value path un-folds it through the projection weights:
        T'[b,d] = sum_s e_s xq[s,d] = qk_d * T[b,d]
        out[b]  = (T'/L) @ (Wv^T / qk[:,None]) + bv
    (scores are O(0.01) here, exp needs no max-subtraction; sum w = 1 moves
    the value projection after the pooling.)

Device layout per core (256 batches), engines balanced:
  tokens on partitions, 128-token groups; xq tiles [128, 8b, 4g, 128d]
  (host-pretransposed, fully contiguous DMA).  Scores = segmented sum:
  2x-mode bf16 halving adds split DVE/GpSimd (h1 DVE, h2 GpSimd, h3 DVE)
  + tensor_reduce(axis=X) + possum add on DVE; exp on ACT.  Pooling on PE
  with e as the (tiny, 4-column) stationary operand against a block-diagonal
  rhs; the [4, 512] PSUM block goes to SBUF once per 4 batches (plain copy)
  and each batch's diagonal [1, 128] block is lifted into a [d, 128b] PSUM
  accumulator as a column via a PE matmul against a basis vector.  The
  softmax denominator L never touches the per-iter chain: GpSimd
  partition_all_reduce of e gives L[b] replicated on every partition, and
  one per-128-batch block epilogue applies 1/L on the transposed projection
  (out.T = wvt'^T @ TsT, scaled columnwise, transposed back, biased).
"""

import numpy as np
import ml_dtypes

TOKEN_DIM = 128
SCALE = TOKEN_DIM ** -0.5
B, S, D = 2048, 512, 128
NCORES = 8
BSH = B // NCORES          # 256 batches per core
G = S // 128               # 4 token groups of 128 per batch
BB = 8                     # batches per DMA/DVE super-iteration
NSUP = BSH // BB           # 32 super-iterations per core
SUB = BB // 4              # PE sub-iterations (4 batches each) per super-iter
BLK = 128                  # batches per output block (projection granularity)
SUPS_PER_BLK = BLK // BB   # 16
NBLK = BSH // BLK          # 2

_CACHE = {}


def _split_multi_waits(nc):
    """The walrus build here rejects instructions carrying more than one
    semaphore wait (limit varies by ISA struct; STT and Drain allow 1).
    Hoist extra waits onto same-engine NoOps placed just before the
    instruction — identical blocking semantics, trivial cost."""
    from concourse import mybir

    n = 0
    for f in nc.m.functions:
        for bb in f.blocks:
            new = []
            for inst in bb.instructions:
                si = inst.sync_info
                if si is not None and si.on_wait and len(si.on_wait) > 1:
                    waits = list(si.on_wait)
                    for w in waits[1:]:
                        n += 1
                        nop = mybir.InstNoOp(
                            name=f"T-wsplit-{n}", engine=inst.engine, ins=[], outs=[]
                        )
                        nop.sync_info = mybir.SyncInfo(on_wait=[w], on_update=[])
                        new.append(nop)
                    inst.sync_info = mybir.SyncInfo(
                        on_wait=[waits[0]], on_update=list(si.on_update or [])
                    )
                new.append(inst)
            bb.instructions = new
    return n


def build_program():
    """Build the per-core Bass program (SPMD across the 8 cores)."""
    import concourse.bass as bass
    import concourse.tile as tile
    from concourse import mybir, bass_isa

    f32 = mybir.dt.float32
    bf16 = mybir.dt.bfloat16
    Exp = mybir.ActivationFunctionType.Exp
    Copy = mybir.ActivationFunctionType.Copy
    Add = mybir.AluOpType.add
    AxX = mybir.AxisListType.X

    nc = bass.Bass("TRN2", target_bir_lowering=False, debug=False)
    x_d = nc.dram_tensor("x", [NSUP, 128, BB, G, D], bf16, kind="ExternalInput").ap()
    possum_d = nc.dram_tensor("possum", [128, BSH, G], f32, kind="ExternalInput").ap()
    wvt_d = nc.dram_tensor("wvt", [D, D], bf16, kind="ExternalInput").ap()
    bvb_d = nc.dram_tensor("bvb", [128, D], f32, kind="ExternalInput").ap()
    ident_d = nc.dram_tensor("ident", [128, 128], bf16, kind="ExternalInput").ap()
    out_d = nc.dram_tensor("out", [BSH, D], f32, kind="ExternalOutput").ap()

    from contextlib import ExitStack

    with tile.TileContext(nc) as tc, ExitStack() as ctx:
            pool = lambda name, bufs, **kw: ctx.enter_context(
                tc.tile_pool(name=name, bufs=bufs, **kw)
            )
            consts = pool("consts", 1)
            xin_pool = pool("xin", 5)
            h1_pool = pool("h1", 3)
            h2_pool = pool("h2", 3)
            h3_pool = pool("h3", 2)
            sx_pool = pool("sx", 2)
            sc_pool = pool("sc", 2)
            e_pool = pool("e", 3)
            ep_pool = pool("ep", 2)
            lp_pool = pool("lpsum", 1, space="PSUM")
            lsb_pool = pool("lsb", 2)
            lt_pool = pool("ltpsum", 1, space="PSUM")
            tsb_pool = pool("tsb", 4)
            tpsum_pool = pool("tpsum", 2, space="PSUM")
            ttpsum_pool = pool("ttpsum", 2, space="PSUM")
            pj_pool = pool("pjpsum", 1, space="PSUM")
            ob_pool = pool("obpsum", 1, space="PSUM")
            rcpb_pool = pool("rcpb", 2)
            tst_pool = pool("tst", 2)
            s1_pool = pool("s1", 2)
            osb_pool = pool("osb", 2)
            possum_sb = consts.tile([128, BSH, G], f32)
            nc.sync.dma_start(possum_sb[:], possum_d[:])
            wvt_sb = consts.tile([D, D], bf16)
            nc.sync.dma_start(wvt_sb[:], wvt_d[:])
            bvb_sb = consts.tile([128, D], f32)
            nc.sync.dma_start(bvb_sb[:], bvb_d[:])
            ident_sb = consts.tile([128, 128], bf16)
            nc.sync.dma_start(ident_sb[:], ident_d[:])
            onesf_sb = consts.tile([128, 1], f32)
            nc.vector.memset(onesf_sb[:], 1.0)

            for blk in range(NBLK):
                # [d, b] accumulator for this 128-batch block, one column per
                # batch from the basis matmuls; L replicated on all partitions.
                tt = ttpsum_pool.tile([128, BLK], f32, tag="tt")
                lt = lt_pool.tile([128, BLK], f32, tag="lt")
                for sup_i in range(SUPS_PER_BLK):
                    sup = blk * SUPS_PER_BLK + sup_i
                    b0 = sup * BB
                    xin = xin_pool.tile([128, BB, G, D], bf16)
                    nc.sync.dma_start(xin[:], x_d[sup])
                    # scores: sum_d xq via 2x-mode halving adds + reduce
                    h1 = h1_pool.tile([128, BB, G, D // 2], bf16)
                    nc.vector.tensor_add(
                        h1[:], xin[:, :, :, 0 : D // 2], xin[:, :, :, D // 2 : D]
                    )
                    h2 = h2_pool.tile([128, BB, G, D // 4], bf16)
                    nc.gpsimd.tensor_add(
                        h2[:], h1[:, :, :, 0 : D // 4], h1[:, :, :, D // 4 : D // 2]
                    )
                    h3 = h3_pool.tile([128, BB, G, D // 8], bf16)
                    nc.vector.tensor_add(
                        h3[:], h2[:, :, :, 0 : D // 8], h2[:, :, :, D // 8 : D // 4]
                    )
                    sx = sx_pool.tile([128, BB, G], f32)
                    nc.vector.tensor_reduce(sx[:], h3[:], axis=AxX, op=Add)
                    sc = sc_pool.tile([128, BB, G], f32)
                    nc.vector.tensor_add(sc[:], sx[:], possum_sb[:, b0 : b0 + BB, :])
                    e = e_pool.tile([128, BB, G], bf16)
                    nc.scalar.activation(e[:], sc[:], Exp)
                    # softmax denominators: L[b] = sum_p ep[p, b] via a tiny
                    # fp32 matmul, then replicated onto every partition with a
                    # broadcast-weights matmul (engines cannot cross lanes,
                    # the PE can).
                    ep = ep_pool.tile([128, BB], f32)
                    nc.vector.tensor_reduce(ep[:], e[:], axis=AxX, op=Add)
                    lp = lp_pool.tile([BB, 1], f32, tag="lp")
                    nc.tensor.matmul(
                        out=lp[:], lhsT=ep[:], rhs=onesf_sb[:], start=True, stop=True
                    )
                    lsb = lsb_pool.tile([BB, 1], bf16, tag="lsb")
                    nc.scalar.activation(lsb[:], lp[:], Copy)
                    nc.tensor.matmul(
                        out=lt[:, sup_i * BB : (sup_i + 1) * BB],
                        lhsT=lsb[:].broadcast_to((BB, 128)),
                        rhs=ident_sb[0:BB, 0:BB],
                        start=True,
                        stop=True,
                    )

                    for s in range(SUB):
                        itb = sup_i * SUB + s       # 4-batch index in block
                        bs = 4 * s
                        tp = tpsum_pool.tile([4, 4 * D], f32, tag="tp")
                        for g in range(G):
                            nc.tensor.matmul(
                                out=tp[:],
                                lhsT=e[:, bs : bs + 4, g],
                                rhs=xin[:, bs : bs + 4, g, :],
                                start=(g == 0),
                                stop=(g == G - 1),
                            )
                        # whole [4, 512] block to SBUF (unscaled)
                        tsb = tsb_pool.tile([4, 4 * D], bf16, tag="tsb")
                        nc.scalar.activation(tsb[:], tp[:], Copy)
                        # lift each diagonal [1, D] row into tt as a column:
                        # chunk_b^T @ basis_b = row b of chunk_b = T'_b
                        for bb in range(4):
                            nc.tensor.matmul(
                                out=tt[:, itb * 4 + bb : itb * 4 + bb + 1],
                                lhsT=tsb[:, bb * D : (bb + 1) * D],
                                rhs=ident_sb[0:4, bb : bb + 1],
                                start=True,
                                stop=True,
                            )
                # block epilogue: transposed projection, 1/L columnwise,
                # transpose back, bias, store
                rcpb = rcpb_pool.tile([128, BLK], f32, tag="rcpb")
                nc.vector.reciprocal(rcpb[:], lt[:])
                tst = tst_pool.tile([128, BLK], bf16, tag="tst")
                nc.scalar.activation(tst[:], tt[:], Copy)
                pj = pj_pool.tile([BLK, D], f32, tag="pj")
                nc.tensor.matmul(
                    out=pj[:], lhsT=wvt_sb[:], rhs=tst[:], start=True, stop=True
                )
                s1 = s1_pool.tile([BLK, D], bf16, tag="s1")
                nc.vector.tensor_mul(s1[:], pj[:], rcpb[:])
                ob = ob_pool.tile([BLK, D], bf16, tag="ob")
                nc.tensor.transpose(ob[:], s1[:], ident_sb[:])
                osb = osb_pool.tile([BLK, D], f32, tag="osb")
                nc.vector.tensor_add(osb[:], ob[:], bvb_sb[:])
                nc.sync.dma_start(out_d[blk * BLK : (blk + 1) * BLK, :], osb[:])

    _split_multi_waits(nc)
    return nc


def prepare_inputs(input_features, positions, mask, query, Wk, bk, Wv, bv, Wp, bp):
    """Host-side prep: fold the query into the projections, shard on batch."""
    bf = ml_dtypes.bfloat16
    q = np.asarray(query, np.float32)[0]
    qk = (q @ np.asarray(Wk, np.float32)) * SCALE            # [D]
    qp = (q @ np.asarray(Wp, np.float32)) * SCALE            # [4]

    x = np.asarray(input_features, np.float32)
    xq = (x * qk[None, None, :]).astype(bf)                  # [B, S, D]
    # per-core, host-pretransposed tile layout [sup, p, bb, g, d]
    xr = np.ascontiguousarray(
        xq.reshape(NCORES, NSUP, BB, G, 128, D).transpose(0, 1, 4, 2, 3, 5)
    )

    # possum[p, b, g] = pos[b, 128g+p, :] . qp  (masked tokens -> -1e30 so
    # their softmax weight underflows to exactly 0)
    possum = np.asarray(positions, np.float32) @ qp          # [B, S]
    m = np.asarray(mask, bool)
    if not m.all():
        possum = np.where(m, possum, np.float32(-1e30))
    possum = np.ascontiguousarray(
        possum.reshape(B, G, 128).transpose(2, 0, 1), np.float32
    )                                                         # [128, B, G]

    wvt = np.ascontiguousarray(
        (np.asarray(Wv, np.float32).T / qk[:, None]).astype(bf)
    )                                                         # [d, j] = Wv[j,d]/qk_d
    bvb = np.ascontiguousarray(
        np.broadcast_to(np.asarray(bv, np.float32)[None, :], (128, D))
    )
    ident = np.eye(128, dtype=bf)

    in_maps = []
    for c in range(NCORES):
        in_maps.append(
            {
                "x": xr[c],
                "possum": np.ascontiguousarray(possum[:, c * BSH : (c + 1) * BSH]),
                "wvt": wvt,
                "bvb": bvb,
                "ident": ident,
            }
        )
    return in_maps


def kernel(input_features, positions, mask, query, Wk, bk, Wv, bv, Wp, bp):
    from concourse.bass_utils import run_bass_kernel_spmd

    if "nc" not in _CACHE:
        _CACHE["nc"] = build_program()
    nc = _CACHE["nc"]
    in_maps = prepare_inputs(
        input_features, positions, mask, query, Wk, bk, Wv, bv, Wp, bp
    )
    res = run_bass_kernel_spmd(nc, in_maps, list(range(NCORES)))
    return np.concatenate([res.results[c]["out"] for c in range(NCORES)], axis=0)
